# revision 2
# baseline (speedup 1.0000x reference)
"""MoE layer (top-2 routing, E=8 experts) on 8 Trainium2 NeuronCores.

Strategy (expert parallelism, per the sharding hint):
  - Host computes the gate (T x 8 logits -> top-2 -> softmax) and dispatches
    each token to its two routed experts ("all-to-all" realized as host-side
    sharding, since kernel() receives full inputs and returns full output).
  - Core e owns expert e's weights and runs a dense FFN
    relu(Xe @ w1[e]) @ w2[e], scaled by the per-token gate, over the <=C
    tokens routed to expert e (zero-padded to capacity C).
  - Host scatter-adds the 8 per-expert results back into [B, S, D].

The FFN is computed as two GEMM phases on the PE array in fp16 (fp32 PSUM
accumulation), keeping H^T = relu(W1^T X^T) tile-blocked in DRAM between
phases:
  phase A: H^T[h, c] = relu( sum_k W1[k, h]^T X^T[k, c] )   (lhsT = W1 tile)
  phase B: Y[c, d]   = gate[c] * sum_k H^T[k, c]^T W2[k, d] (lhsT = H^T tile)
"""

import numpy as np

B, S, D, E = 4, 2048, 1024, 8
H = 4 * D
T = B * S
TOP_K = 2
P = 128
NT = 512  # matmul moving free dim / PSUM bank
C_DEFAULT = 2560  # capacity per expert; actual max load for these inputs is ~2182

_compiled = {}  # C -> compiled Bacc program


def _build(C):
    import concourse.mybir as mybir
    import concourse.tile as tile
    from concourse import bacc

    KA = D // P   # 8   contraction tiles, phase A
    MA = H // P   # 32  h tiles (output partitions), phase A
    CA = C // NT  #     token chunks, phase A
    KB = H // P   # 32  contraction tiles, phase B
    MB = C // P   #     token tiles (output partitions), phase B
    NB = D // NT  # 2   output chunks, phase B

    fp16 = mybir.dt.float16
    fp32 = mybir.dt.float32

    nc = bacc.Bacc("TRN2", target_bir_lowering=False, debug=False, num_devices=E)

    # w1t: [MA, KA, P, P] tile-blocked W1 (w1[e].reshape(KA,P,MA,P).transpose(2,0,1,3))
    # w2t: [KB, P, D] = w1[e] reshaped (rows already contiguous)
    # xt:  [D, C] = gathered tokens, transposed
    # gate: [MB, P, 1] per-token combine weight
    xt = nc.dram_tensor("xt", [KA, P, C], fp16, kind="ExternalInput")
    w1t = nc.dram_tensor("w1t", [MA, KA, P, P], fp16, kind="ExternalInput")
    w2t = nc.dram_tensor("w2t", [KB, P, D], fp16, kind="ExternalInput")
    gate = nc.dram_tensor("gate", [MB, P, 1], fp32, kind="ExternalInput")
    y = nc.dram_tensor("y", [C, D], fp32, kind="ExternalOutput")
    # H^T scratch, tile-blocked: ht[m_token_tile, k_h_tile, P(h), P(token)]
    ht = nc.dram_tensor("ht", [MB, KB, P, P], fp16)

    with tile.TileContext(nc) as tc:
        # ---- Phase A: H^T = relu(W1^T @ X^T) ----
        with tc.tile_pool(name="xt_pool", bufs=1) as xtp, \
             tc.tile_pool(name="w1_pool", bufs=3) as w1p, \
             tc.tile_pool(name="h_pool", bufs=4) as hp, \
             tc.tile_pool(name="psA", bufs=4, space="PSUM") as psA:
            xt_sb = xtp.tile([P, KA * C], fp16)
            nc.sync.dma_start(
                xt_sb[:].rearrange("p (k c) -> p k c", k=KA),
                xt[:].rearrange("k p c -> p k c")
            )
            for m in range(MA):
                w1_sb = w1p.tile([P, KA * P], fp16)
                nc.sync.dma_start(
                    w1_sb[:].rearrange("p (k q) -> p k q", k=KA),
                    w1t[m].rearrange("k p q -> p k q")
                )
                for c in range(CA):
                    ps = psA.tile([P, NT], fp32)
                    for k in range(KA):
                        nc.tensor.matmul(
                            ps[:],
                            w1_sb[:, k * P:(k + 1) * P],
                            xt_sb[:, k * C + c * NT: k * C + c * NT + NT],
                            start=(k == 0),
                            stop=(k == KA - 1),
                        )
                    h_sb = hp.tile([P, NT], fp16)
                    nc.scalar.activation(
                        h_sb[:], ps[:], mybir.ActivationFunctionType.Relu
                    )
                    # scatter the 512-token strip into 4 token-tile blocks
                    nc.sync.dma_start(
                        ht[c * (NT // P):(c + 1) * (NT // P), m].rearrange(
                            "f p q -> p f q"
                        ),
                        h_sb[:].rearrange("p (f q) -> p f q", f=NT // P),
                    )

        # ---- Phase B: Y = gate * (H @ W2) ----
        with tc.tile_pool(name="w2_pool", bufs=1) as w2p, \
             tc.tile_pool(name="ht_pool", bufs=3) as htp, \
             tc.tile_pool(name="g_pool", bufs=1) as gp, \
             tc.tile_pool(name="y_pool", bufs=4) as yp, \
             tc.tile_pool(name="psB", bufs=4, space="PSUM") as psB:
            w2_sb = w2p.tile([P, KB * D], fp16)
            nc.sync.dma_start(
                w2_sb[:].rearrange("p (k d) -> p k d", k=KB),
                w2t[:].rearrange("k p d -> p k d")
            )
            gate_sb = gp.tile([P, MB], fp32)
            nc.sync.dma_start(
                gate_sb[:].rearrange("p (m o) -> p m o", o=1),
                gate[:].rearrange("m p o -> p m o")
            )
            for m in range(MB):
                ht_sb = htp.tile([P, KB * P], fp16)
                nc.sync.dma_start(
                    ht_sb[:].rearrange("p (k q) -> p k q", k=KB),
                    ht[m].rearrange("k p q -> p k q")
                )
                for n in range(NB):
                    ps = psB.tile([P, NT], fp32)
                    for k in range(KB):
                        nc.tensor.matmul(
                            ps[:],
                            ht_sb[:, k * P:(k + 1) * P],
                            w2_sb[:, k * D + n * NT: k * D + n * NT + NT],
                            start=(k == 0),
                            stop=(k == KB - 1),
                        )
                    y_sb = yp.tile([P, NT], fp32)
                    nc.vector.tensor_scalar_mul(
                        y_sb[:], ps[:], gate_sb[:, m:m + 1]
                    )
                    nc.sync.dma_start(
                        y[m * P:(m + 1) * P, n * NT:(n + 1) * NT], y_sb[:]
                    )

    nc.compile()
    return nc


def _get_program(C):
    if C not in _compiled:
        _compiled[C] = _build(C)
    return _compiled[C]


def _route(x2d, w_gate):
    """Top-2 routing + softmax on host. Returns (idx1, idx2, g1, g2)."""
    logits = x2d @ w_gate  # [T, E] fp32
    i1 = np.argmax(logits, axis=1)
    rows = np.arange(logits.shape[0])
    l1 = logits[rows, i1]
    masked = logits.copy()
    masked[rows, i1] = -np.inf
    i2 = np.argmax(masked, axis=1)
    l2 = masked[rows, i2]
    # softmax over the two selected logits
    z = np.exp((l2 - l1).astype(np.float64))
    g1 = (1.0 / (1.0 + z)).astype(np.float32)
    g2 = (z / (1.0 + z)).astype(np.float32)
    return i1, i2, g1, g2


def kernel(x, w_gate, w1, w2, _want_results=False, _run_kwargs=None):
    from concourse.bass_utils import run_bass_kernel_spmd

    x = np.asarray(x, dtype=np.float32)
    w_gate = np.asarray(w_gate, dtype=np.float32)
    w1 = np.asarray(w1, dtype=np.float32)
    w2 = np.asarray(w2, dtype=np.float32)

    x2d = x.reshape(-1, D)
    i1, i2, g1, g2 = _route(x2d, w_gate)

    # token lists per expert
    idx_e = []
    gate_e = []
    for e in range(E):
        m1 = np.nonzero(i1 == e)[0]
        m2 = np.nonzero(i2 == e)[0]
        idx = np.concatenate([m1, m2])
        gv = np.concatenate([g1[m1], g2[m2]])
        idx_e.append(idx)
        gate_e.append(gv)
    max_load = max(len(i) for i in idx_e)
    C = max(C_DEFAULT, -(-max_load // NT) * NT)

    nc = _get_program(C)

    xt_full = np.ascontiguousarray(x2d.T.astype(np.float16))  # [D, T]
    KA, MA, KB, MB = D // P, H // P, H // P, C // P

    in_maps = []
    for e in range(E):
        n_e = len(idx_e[e])
        xt_e = np.zeros((D, C), dtype=np.float16)
        xt_e[:, :n_e] = xt_full[:, idx_e[e]]
        gate_arr = np.zeros((C,), dtype=np.float32)
        gate_arr[:n_e] = gate_e[e]
        w1_e = w1[e].astype(np.float16)
        w2_e = w2[e].astype(np.float16)
        in_maps.append({
            "xt": np.ascontiguousarray(xt_e.reshape(KA, P, C)),
            "w1t": np.ascontiguousarray(
                w1_e.reshape(KA, P, MA, P).transpose(2, 0, 1, 3)
            ),
            "w2t": np.ascontiguousarray(w2_e.reshape(KB, P, D)),
            "gate": np.ascontiguousarray(gate_arr.reshape(MB, P, 1)),
        })

    res = run_bass_kernel_spmd(
        nc, in_maps, list(range(E)), **(_run_kwargs or {})
    )

    out = np.zeros((T, D), dtype=np.float32)
    for e in range(E):
        n_e = len(idx_e[e])
        y_e = res.results[e]["y"]
        out[idx_e[e]] += y_e[:n_e]

    if _want_results:
        return out.reshape(B, S, D), res
    return out.reshape(B, S, D)


# revision 3
# speedup vs baseline: 1.0877x; 1.0877x over previous
"""MoE layer (top-2 routing, E=8 experts) on 8 Trainium2 NeuronCores.

Strategy (expert parallelism, per the sharding hint):
  - Host computes the gate (T x 8 logits -> top-2 -> softmax) and dispatches
    each token to its two routed experts ("all-to-all" realized as host-side
    sharding, since kernel() receives full inputs and returns full output).
  - Core e owns expert e's weights and runs a dense FFN
    relu(Xe @ w1[e]) @ w2[e], scaled by the per-token gate, over the <=C
    tokens routed to expert e (zero-padded to capacity C).
  - Host scatter-adds the 8 per-expert results back into [B, S, D].

The FFN runs fully fused per 512-token chunk, fp16 operands with fp32 PSUM
accumulation, H^T chunk kept in SBUF between the two GEMMs:
  GEMM1: H^T[h, c] = relu( sum_k W1[k, h]^T X^T[k, c] )   (lhsT = W1 tile)
  GEMM2: Y[c, d]   = gate[c] * sum_k H^T[k, c]^T W2[k, d] (lhsT = H^T tile)
"""

import numpy as np

B, S, D, E = 4, 2048, 1024, 8
H = 4 * D
T = B * S
TOP_K = 2
P = 128
NT = 512  # matmul moving free dim / PSUM bank
C_DEFAULT = 2304  # capacity per expert (mult of 128); actual max load ~2182

_compiled = {}  # C -> compiled Bacc program


def _chunks(C):
    out = []
    off = 0
    while off < C:
        w = min(NT, C - off)
        out.append((off, w))
        off += w
    return out


def _build(C):
    import concourse.mybir as mybir
    import concourse.tile as tile
    from concourse import bacc

    assert C % P == 0
    KA = D // P   # 8   contraction tiles, GEMM1
    MA = H // P   # 32  h tiles (GEMM1 output partitions) == GEMM2 k tiles
    KB = H // P   # 32
    MB = C // P   # token tiles
    NB = D // NT  # 2   output chunks, GEMM2

    fp16 = mybir.dt.float16
    fp32 = mybir.dt.float32

    nc = bacc.Bacc("TRN2", target_bir_lowering=False, debug=False, num_devices=E)

    xt = nc.dram_tensor("xt", [KA, P, C], fp16, kind="ExternalInput")
    w1t = nc.dram_tensor("w1t", [MA, KA, P, P], fp16, kind="ExternalInput")
    w2t = nc.dram_tensor("w2t", [KB, P, D], fp16, kind="ExternalInput")
    gate = nc.dram_tensor("gate", [MB, P, 1], fp32, kind="ExternalInput")
    y = nc.dram_tensor("y", [C, D], fp32, kind="ExternalOutput")

    with tile.TileContext(nc) as tc:
        with tc.tile_pool(name="xt_pool", bufs=1) as xtp, \
             tc.tile_pool(name="w1_pool", bufs=3) as w1p, \
             tc.tile_pool(name="w2_pool", bufs=1) as w2p, \
             tc.tile_pool(name="h_pool", bufs=2) as hp, \
             tc.tile_pool(name="g_pool", bufs=1) as gp, \
             tc.tile_pool(name="y_pool", bufs=4) as yp, \
             tc.tile_pool(name="psA", bufs=4, space="PSUM") as psA, \
             tc.tile_pool(name="psB", bufs=4, space="PSUM") as psB:

            # resident tensors, loaded once up front
            w2_sb = w2p.tile([P, KB * D], fp16)
            nc.sync.dma_start(
                w2_sb[:].rearrange("p (k d) -> p k d", k=KB),
                w2t[:].rearrange("k p d -> p k d"),
            )
            gate_sb = gp.tile([P, MB], fp32)
            nc.sync.dma_start(
                gate_sb[:].rearrange("p (m o) -> p m o", o=1),
                gate[:].rearrange("m p o -> p m o"),
            )
            xt_sb = xtp.tile([P, KA * C], fp16)
            for k in range(KA):
                nc.sync.dma_start(xt_sb[:, k * C:(k + 1) * C], xt[k])

            for coff, cw in _chunks(C):
                # ---- GEMM1 for this chunk: H^T[:, coff:coff+cw] in SBUF ----
                h_sb = hp.tile([P, MA * NT], fp16, tag="hchunk")
                for m in range(MA):
                    w1_sb = w1p.tile([P, KA * P], fp16)
                    nc.sync.dma_start(
                        w1_sb[:].rearrange("p (k q) -> p k q", k=KA),
                        w1t[m].rearrange("k p q -> p k q"),
                    )
                    ps = psA.tile([P, NT], fp32, tag="psA")
                    for k in range(KA):
                        nc.tensor.matmul(
                            ps[:, :cw],
                            w1_sb[:, k * P:(k + 1) * P],
                            xt_sb[:, k * C + coff: k * C + coff + cw],
                            start=(k == 0),
                            stop=(k == KA - 1),
                        )
                    nc.scalar.activation(
                        h_sb[:, m * cw:(m + 1) * cw], ps[:, :cw],
                        mybir.ActivationFunctionType.Relu,
                    )

                # ---- GEMM2 for this chunk ----
                for mt in range(cw // P):
                    tok = coff // P + mt
                    for n in range(NB):
                        ps2 = psB.tile([P, NT], fp32, tag="psB")
                        for k in range(KB):
                            nc.tensor.matmul(
                                ps2[:],
                                h_sb[:, k * cw + mt * P: k * cw + (mt + 1) * P],
                                w2_sb[:, k * D + n * NT: k * D + (n + 1) * NT],
                                start=(k == 0),
                                stop=(k == KB - 1),
                            )
                        y_sb = yp.tile([P, NT], fp32)
                        nc.vector.tensor_scalar_mul(
                            y_sb[:], ps2[:], gate_sb[:, tok:tok + 1]
                        )
                        nc.sync.dma_start(
                            y[tok * P:(tok + 1) * P, n * NT:(n + 1) * NT],
                            y_sb[:],
                        )

    nc.compile()
    return nc


def _get_program(C):
    if C not in _compiled:
        _compiled[C] = _build(C)
    return _compiled[C]


def _route(x2d, w_gate):
    """Top-2 routing + softmax on host. Returns (idx1, idx2, g1, g2)."""
    logits = x2d @ w_gate  # [T, E] fp32
    i1 = np.argmax(logits, axis=1)
    rows = np.arange(logits.shape[0])
    l1 = logits[rows, i1]
    masked = logits.copy()
    masked[rows, i1] = -np.inf
    i2 = np.argmax(masked, axis=1)
    l2 = masked[rows, i2]
    # softmax over the two selected logits
    z = np.exp((l2 - l1).astype(np.float64))
    g1 = (1.0 / (1.0 + z)).astype(np.float32)
    g2 = (z / (1.0 + z)).astype(np.float32)
    return i1, i2, g1, g2


def kernel(x, w_gate, w1, w2, _want_results=False, _run_kwargs=None):
    from concourse.bass_utils import run_bass_kernel_spmd

    x = np.asarray(x, dtype=np.float32)
    w_gate = np.asarray(w_gate, dtype=np.float32)
    w1 = np.asarray(w1, dtype=np.float32)
    w2 = np.asarray(w2, dtype=np.float32)

    x2d = x.reshape(-1, D)
    i1, i2, g1, g2 = _route(x2d, w_gate)

    # token lists per expert
    idx_e = []
    gate_e = []
    for e in range(E):
        m1 = np.nonzero(i1 == e)[0]
        m2 = np.nonzero(i2 == e)[0]
        idx_e.append(np.concatenate([m1, m2]))
        gate_e.append(np.concatenate([g1[m1], g2[m2]]))
    max_load = max(len(i) for i in idx_e)
    C = max(C_DEFAULT, -(-max_load // P) * P)

    nc = _get_program(C)

    xt_full = np.ascontiguousarray(x2d.T.astype(np.float16))  # [D, T]
    KA, MA, KB, MB = D // P, H // P, H // P, C // P

    in_maps = []
    for e in range(E):
        n_e = len(idx_e[e])
        xt_e = np.zeros((D, C), dtype=np.float16)
        xt_e[:, :n_e] = xt_full[:, idx_e[e]]
        gate_arr = np.zeros((C,), dtype=np.float32)
        gate_arr[:n_e] = gate_e[e]
        w1_e = w1[e].astype(np.float16)
        w2_e = w2[e].astype(np.float16)
        in_maps.append({
            "xt": np.ascontiguousarray(xt_e.reshape(KA, P, C)),
            "w1t": np.ascontiguousarray(
                w1_e.reshape(KA, P, MA, P).transpose(2, 0, 1, 3)
            ),
            "w2t": np.ascontiguousarray(w2_e.reshape(KB, P, D)),
            "gate": np.ascontiguousarray(gate_arr.reshape(MB, P, 1)),
        })

    res = run_bass_kernel_spmd(
        nc, in_maps, list(range(E)), **(_run_kwargs or {})
    )

    out = np.zeros((T, D), dtype=np.float32)
    for e in range(E):
        n_e = len(idx_e[e])
        y_e = res.results[e]["y"]
        out[idx_e[e]] += y_e[:n_e]

    if _want_results:
        return out.reshape(B, S, D), res
    return out.reshape(B, S, D)
